# revision 2
# baseline (speedup 1.0000x reference)
"""GAT layer on trn2, v5: engine-cost-aware minimal-instruction design.

Measured per-instruction costs in this environment (8 cores):
  PE matmul/transpose  ~36us      (regardless of size or accumulation)
  ACT activation       ~39us
  DVE ops              ~0.03-2.4us (essentially free for big tiles)
  gpsimd C-reduce      ~278us     (!)
  gpsimd tensor op     ~12us
  DMA                  ~60-110us each; one big load 83us
  cross-engine dep hop ~52us

So v5 pushes everything possible onto DVE, eliminates gpsimd entirely,
batches ACT ops, and keeps PE at the ISA minimum:
  - 1 input DMA (blob with host-transposed adj), 1 output store
  - PE: 9 setup + 16 R-tile + 64 accumulation + 8 broadcast matmuls = 97
  - partition-axis sums (softmax denominator D, layernorm stats) are
    computed as DVE log2 reduction trees, not gpsimd C-reductions
  - per-tile sigma/r factors batched into two [128,16] ACT exps

Math (rho=e^{0.8 Wh1_i}, sigma=e^{Wh2_j}, r=e^{0.2 Wh2_j}):
  w[j,i] = adj[i,j] * max(sigma_j*rho_i, r_j)
  y[o,i] = sum_j Wh[j,o]*w[j,i];  D[i] = sum_j w[j,i]
  out[i,:] = leaky((y - mean_o y)/sqrt(var_o y + eps*D^2))
"""
import sys

sys.path.insert(0, "/opt/trn_rl_repo")

import numpy as np

import concourse.bass as bass
import concourse.mybir as mybir
import concourse.tile as tile
from concourse.bass_utils import run_bass_kernel_spmd

dt = mybir.dt
A = mybir.ActivationFunctionType
Op = mybir.AluOpType

N = 2048
F = 128
T = N // 128      # 16
EPS = 1e-5
ALPHA = 0.2
N_CORES = 8

# blob column map (int32 units)
ADJ_C = N * T                  # 0     : 32768  adj^T
HT_C = ADJ_C                   # 32768 : 34816  hT (f32) [f, i]
WWT_C = HT_C + N               # 34816 : 34944  Ww^T (f32) [f, o]
WWN_C = WWT_C + F              # 34944 : 35072  Ww natural (f32) [o, f]
AC_C = WWN_C + F               # 35072 : 35074  acols (f32) [o, 2]
A2B_C = AC_C + 2               # 35074 : 35138  a2 bcast (fp16) [*, o]
WBB_C = A2B_C + F // 2         # 35138 : 35202  W_b bcast (fp16) [*, o]
AR_C = WBB_C + F // 2          # 35202 : 35330  a rows (f32) [2, o]
WB2_C = AR_C + F               # 35330 : 35458  W_b rows (f32) [2, o]
BLOB_C = WB2_C + F + 2         # 35460


def _fix_sync_waits(nc, max_waits=1):
    """walrus here rejects >1 sync wait per instruction; spill extras onto
    same-engine no-ops inserted just before."""
    k = 0
    for f in nc.m.functions:
        for blk in f.blocks:
            insts = blk.instructions
            i = 0
            while i < len(insts):
                inst = insts[i]
                si = inst.sync_info
                if si is not None and len(si.on_wait) > max_waits:
                    waits = list(si.on_wait)
                    inst.sync_info = mybir.SyncInfo(
                        on_wait=waits[:max_waits], on_update=list(si.on_update))
                    pos = i
                    for w in waits[max_waits:]:
                        nop = mybir.InstNoOp(name=f"wait_spill_{k}", ins=[], outs=[])
                        k += 1
                        nop.engine = inst.engine
                        nop.sync_info = mybir.SyncInfo(on_wait=[w], on_update=[])
                        insts.insert(pos, nop)
                        pos += 1
                        i += 1
                i += 1


def build_gat_nc(reps=1, fix_waits=True, internal=False):
    nc = bass.Bass()
    if internal:
        BLOB = nc.dram_tensor("blob", [128, BLOB_C], dt.int32, kind="Internal")
        DUM = nc.dram_tensor("dum", [1, 64], dt.int32, kind="ExternalInput")
    OUT = nc.dram_tensor("out", [128, N], dt.float16, kind="ExternalOutput")
    if not internal:
        BLOB = nc.dram_tensor("blob", [128, BLOB_C], dt.int32,
                              kind="ExternalInput")

    with tile.TileContext(nc) as tc:
        with tc.tile_pool(name="const", bufs=1) as const:
            ones_h = const.tile([1, 128], dt.float16)
            nc.vector.memset(ones_h[:], 1.0)
            ones_c = const.tile([128, 1], dt.float32)
            nc.vector.memset(ones_c[:], 1.0)
            if internal:
                dumt = const.tile([1, 64], dt.int32)
                nc.sync.dma_start(out=dumt[:], in_=DUM[:])

            for _ in range(reps):
                _emit_body(nc, tc, BLOB, OUT, ones_h, ones_c)

    if fix_waits:
        _fix_sync_waits(nc)
    return nc


def _emit_body(nc, tc, BLOB, OUT, ones_h, ones_c):
    with tc.tile_pool(name="keep", bufs=1) as keep:
        y_sb = keep.tile([128, N], dt.float32, tag="ysb", name="ysb")
        wacc = keep.tile([128, N], dt.float32, tag="wacc", name="wacc")

        _emit_main(nc, tc, BLOB, ones_h, y_sb, wacc)
        _emit_epilogue(nc, tc, OUT, ones_h, ones_c, y_sb, wacc)


def _emit_main(nc, tc, BLOB, ones_h, y_sb, wacc):
    with tc.tile_pool(name="blobp", bufs=1) as blobp, \
         tc.tile_pool(name="setup", bufs=1) as setup:

        blob = blobp.tile([128, BLOB_C], dt.int32, tag="blob", name="blob")
        nc.sync.dma_start(out=blob[:], in_=BLOB[:])

        bf = blob[:].bitcast(dt.float32)
        bh = blob[:].bitcast(dt.float16)
        hT_v = bf[:, HT_C:HT_C + N]            # [f, i] f32
        WwT_v = bf[:, WWT_C:WWT_C + F]         # [f, o] f32
        Wwn_v = bf[:, WWN_C:WWN_C + F]         # [o, f] f32
        acols_v = bf[:, AC_C:AC_C + 2]         # [o, 2] f32
        a2b_v = bh[:, 2 * A2B_C:2 * A2B_C + F]     # [*, o] fp16
        wbb_v = bh[:, 2 * WBB_C:2 * WBB_C + F]     # [*, o] fp16
        arows_v = bf[0:2, AR_C:AR_C + F]       # [2, o] f32
        wb2_v = bf[0:2, WB2_C:WB2_C + F]       # [2, o] f32

        with tc.tile_pool(name="spp1", bufs=1, space="PSUM") as spp:
            # ---------------- setup ----------------
            # c_cols[f, r] = sum_o Ww[o, f] * a[r][o]
            pc = spp.tile([128, 2], dt.float32, tag="pc", name="pc")
            nc.tensor.matmul(out=pc[:], lhsT=Wwn_v, rhs=acols_v,
                             start=True, stop=True)
            c_cols = setup.tile([128, 2], dt.float32, tag="cc", name="cc")
            nc.scalar.activation(out=c_cols[:], in_=pc[:], func=A.Identity)

            # beta[r] = sum_o a[r][o] * W_b[o]
            bscr = setup.tile([2, F], dt.float32, tag="bscr", name="bscr")
            beta = setup.tile([2, 1], dt.float32, tag="beta", name="beta")
            nc.vector.tensor_tensor(out=bscr[:], in0=arows_v, in1=wb2_v,
                                    op=Op.mult)
            nc.vector.tensor_reduce(out=beta[:], in_=bscr[:],
                                    axis=mybir.AxisListType.X, op=Op.add)

            # Wh1 row (only row 0 is needed downstream)
            pr = spp.tile([2, N], dt.float32, tag="pr", name="pr")
            for c in range(4):
                nc.tensor.matmul(out=pr[:, c * 512:(c + 1) * 512],
                                 lhsT=c_cols[:],
                                 rhs=hT_v[:, c * 512:(c + 1) * 512],
                                 start=True, stop=True)
            rows2 = setup.tile([2, N], dt.float16, tag="r2", name="r2")
            nc.scalar.activation(out=rows2[:], in_=pr[:], func=A.Identity,
                                 bias=beta[:], scale=1.0)

        with tc.tile_pool(name="spp2", bufs=1, space="PSUM") as spp:
            # rib[*, i] = e^{0.8 Wh1_i} broadcast, fp16
            pb = spp.tile([128, N], dt.float32, tag="pb", name="pb")
            for c in range(4):
                nc.tensor.matmul(out=pb[:, c * 512:(c + 1) * 512],
                                 lhsT=ones_h[0:1, :],
                                 rhs=rows2[0:1, c * 512:(c + 1) * 512],
                                 start=True, stop=True)
            rib = setup.tile([128, N], dt.float16, tag="rib", name="rib")
            nc.scalar.activation(out=rib[:], in_=pb[:], func=A.Exp, scale=0.8)

            # R_all[q, t, o] = Wh[j=t*128+q, o] fp16 (incl. W_b)
            # all 16 matmuls first (consecutive on PE), DVE drains behind
            R_all = setup.tile([128, T, F], dt.float16, tag="R", name="R")
            pRls = []
            for t in range(T):
                pR = spp.tile([128, F], dt.float32, tag="pR", name="pR",
                              bufs=4)
                nc.tensor.matmul(out=pR[:],
                                 lhsT=hT_v[:, t * 128:(t + 1) * 128],
                                 rhs=WwT_v, start=True, stop=True)
                pRls.append(pR)
                if t >= 2:
                    nc.vector.tensor_tensor(out=R_all[:, t - 2, :],
                                            in0=pRls[t - 2][:], in1=wbb_v,
                                            op=Op.add)
            for t in (T - 2, T - 1):
                nc.vector.tensor_tensor(out=R_all[:, t, :], in0=pRls[t][:],
                                        in1=wbb_v, op=Op.add)

            # sigma/r columns for all tiles: Wh2c[q, t] = R_all[q,t,:] . a2
            scr_all = setup.tile([128, T, F], dt.float16, tag="scr",
                                 name="scr")
            for t in range(T):
                nc.vector.tensor_tensor(out=scr_all[:, t, :],
                                        in0=R_all[:, t, :], in1=a2b_v,
                                        op=Op.mult)
            w2c = setup.tile([128, T], dt.float32, tag="w2c", name="w2c")
            nc.vector.tensor_reduce(out=w2c[:], in_=scr_all[:],
                                    axis=mybir.AxisListType.X, op=Op.add)
            sig_all = setup.tile([128, T], dt.float32, tag="sig", name="sig")
            nc.scalar.activation(out=sig_all[:], in_=w2c[:], func=A.Exp,
                                 scale=1.0)
            rj_all = setup.tile([128, T], dt.float32, tag="rj", name="rj")
            nc.scalar.activation(out=rj_all[:], in_=w2c[:], func=A.Exp,
                                 scale=0.2)

        # ---------------- main t-loop ----------------
        with tc.tile_pool(name="mainp", bufs=1) as mainp, \
             tc.tile_pool(name="accp", bufs=1, space="PSUM") as accp:

            acc4 = accp.tile([128, N], dt.float32, tag="acc4", name="acc4")

            for t in range(T):
                # mx[q, i] = max(sigma_j * rho_i, r_j)
                mx = mainp.tile([128, N], dt.float16, tag="mx", name="mx")
                nc.vector.tensor_scalar(out=mx[:], in0=rib[:],
                                        scalar1=sig_all[:, t:t + 1],
                                        scalar2=rj_all[:, t:t + 1],
                                        op0=Op.mult, op1=Op.max)
                # w[q, i] = adjT * mx   (int32 x fp16, exact)
                w = mainp.tile([128, N], dt.float16, tag="w", name="w",
                               bufs=3)
                nc.vector.tensor_tensor(out=w[:],
                                        in0=blob[:, t * N:(t + 1) * N],
                                        in1=mx[:], op=Op.mult)
                # running D partial: wacc[q, i] += w
                if t == 0:
                    nc.vector.tensor_copy(out=wacc[:], in_=w[:])
                else:
                    nc.vector.tensor_tensor(out=wacc[:], in0=wacc[:],
                                            in1=w[:], op=Op.add)
                # y[o, i] += R_t^T @ w
                for c in range(4):
                    nc.tensor.matmul(out=acc4[:, c * 512:(c + 1) * 512],
                                     lhsT=R_all[:, t, :],
                                     rhs=w[:, c * 512:(c + 1) * 512],
                                     start=(t == 0), stop=(t == T - 1))

            # drain y
            nc.scalar.activation(out=y_sb[:], in_=acc4[:], func=A.Identity)


def _emit_epilogue(nc, tc, OUT, ones_h, ones_c, y_sb, wacc):
    with tc.tile_pool(name="epi", bufs=1) as epi:
        # column sums via PE ones-matmuls (DVE reads the PSUM rows directly)
        with tc.tile_pool(name="sumA", bufs=1, space="PSUM") as sumA:
            Dp = sumA.tile([1, N], dt.float32, tag="Dp", name="Dp")
            s1p = sumA.tile([1, N], dt.float32, tag="s1p", name="s1p")
            for c in range(4):
                nc.tensor.matmul(out=Dp[0:1, c * 512:(c + 1) * 512],
                                 lhsT=ones_c[:, 0:1],
                                 rhs=wacc[:, c * 512:(c + 1) * 512],
                                 start=True, stop=True)
                nc.tensor.matmul(out=s1p[0:1, c * 512:(c + 1) * 512],
                                 lhsT=ones_c[:, 0:1],
                                 rhs=y_sb[:, c * 512:(c + 1) * 512],
                                 start=True, stop=True)
            # u = mean, uu = u^2, e2 = eps*D^2
            u = epi.tile([1, N], dt.float32, tag="u", name="u")
            nc.vector.tensor_scalar(out=u[:], in0=s1p[:], scalar1=1.0 / 128,
                                    scalar2=1.0, op0=Op.mult, op1=Op.mult)
            uu = epi.tile([1, N], dt.float32, tag="uu", name="uu")
            nc.vector.tensor_tensor(out=uu[:], in0=u[:], in1=u[:], op=Op.mult)
            Dsb = epi.tile([1, N], dt.float32, tag="Dsb", name="Dsb")
            nc.vector.tensor_copy(out=Dsb[:], in_=Dp[:])
            e2 = epi.tile([1, N], dt.float32, tag="e2", name="e2")
            nc.vector.scalar_tensor_tensor(out=e2[:], in0=Dsb[:], scalar=EPS,
                                           in1=Dsb[:], op0=Op.mult,
                                           op1=Op.mult)

        y2 = epi.tile([128, N], dt.float32, tag="y2", name="y2")
        nc.vector.tensor_tensor(out=y2[:], in0=y_sb[:], in1=y_sb[:],
                                op=Op.mult)
        with tc.tile_pool(name="sumB", bufs=1, space="PSUM") as sumB:
            s2p = sumB.tile([1, N], dt.float32, tag="s2p", name="s2p")
            for c in range(4):
                nc.tensor.matmul(out=s2p[0:1, c * 512:(c + 1) * 512],
                                 lhsT=ones_c[:, 0:1],
                                 rhs=y2[:, c * 512:(c + 1) * 512],
                                 start=True, stop=True)
            var = epi.tile([1, N], dt.float32, tag="y2var", name="var")
            nc.vector.scalar_tensor_tensor(out=var[:], in0=s2p[:],
                                           scalar=1.0 / 128, in1=uu[:],
                                           op0=Op.mult, op1=Op.subtract)

        v2 = epi.tile([1, N], dt.float32, tag="uu", name="v2")
        nc.vector.tensor_tensor(out=v2[:], in0=var[:], in1=e2[:], op=Op.add)
        lnv = epi.tile([1, N], dt.float32, tag="e2", name="lnv")
        nc.scalar.activation(out=lnv[:], in_=v2[:], func=A.Ln)
        rs_row = epi.tile([1, N], dt.float16, tag="rsr", name="rsr")
        nc.scalar.activation(out=rs_row[:], in_=lnv[:], func=A.Exp,
                             scale=-0.5)
        nm_row = epi.tile([1, N], dt.float16, tag="nmr", name="nmr")
        nc.vector.scalar_tensor_tensor(out=nm_row[:], in0=u[:],
                                       scalar=-1.0, in1=rs_row[:],
                                       op0=Op.mult, op1=Op.mult)

        # broadcast rs/nm to [128, N] via PE, then normalize + leaky
        with tc.tile_pool(name="bcp", bufs=1, space="PSUM") as bcp:
            rsb = bcp.tile([128, N], dt.float32, tag="rsb", name="rsb")
            nmb = bcp.tile([128, N], dt.float32, tag="nmb", name="nmb")
            for c in range(4):
                nc.tensor.matmul(out=rsb[:, c * 512:(c + 1) * 512],
                                 lhsT=ones_h[0:1, :],
                                 rhs=rs_row[0:1, c * 512:(c + 1) * 512],
                                 start=True, stop=True)
                nc.tensor.matmul(out=nmb[:, c * 512:(c + 1) * 512],
                                 lhsT=ones_h[0:1, :],
                                 rhs=nm_row[0:1, c * 512:(c + 1) * 512],
                                 start=True, stop=True)
            t1 = epi.tile([128, N], dt.float16, tag="t1", name="t1")
            nc.vector.tensor_tensor(out=t1[:], in0=y_sb[:], in1=rsb[:],
                                    op=Op.mult)
            t2 = epi.tile([128, N], dt.float16, tag="t2", name="t2")
            nc.vector.tensor_tensor(out=t2[:], in0=t1[:], in1=nmb[:],
                                    op=Op.add)
        out_flat = epi.tile([128, N], dt.float16, tag="of", name="of")
        nc.scalar.activation(out=out_flat[:], in_=t2[:], func=A.Prelu,
                             alpha=ALPHA)

        nc.sync.dma_start(out=OUT[:], in_=out_flat[:])


_NC_CACHE = None


def _get_nc():
    global _NC_CACHE
    if _NC_CACHE is None:
        _NC_CACHE = build_gat_nc()
    return _NC_CACHE


def make_blobs(h, adj, W_w, W_b, a_w):
    B = h.shape[0]
    blob = np.zeros((B, 128, BLOB_C), np.int32)
    for b in range(B):
        adjT = np.ascontiguousarray(adj[b].T)
        blob[b, :, 0:ADJ_C] = (
            adjT.reshape(T, 128, N).transpose(1, 0, 2).reshape(128, ADJ_C))
        blob[b, :, HT_C:HT_C + N] = (
            np.ascontiguousarray(h[b].T).view(np.int32))
    blob[:, :, WWT_C:WWT_C + F] = np.ascontiguousarray(W_w.T).view(np.int32)[None]
    blob[:, :, WWN_C:WWN_C + F] = W_w.view(np.int32)[None]
    blob[:, :, AC_C:AC_C + 2] = (
        np.ascontiguousarray(a_w.reshape(2, F).T).view(np.int32)[None])
    a2b = np.tile(a_w[F:].astype(np.float16)[None, :], (128, 1))
    blob[:, :, A2B_C:A2B_C + F // 2] = a2b.view(np.int32)[None]
    wbb = np.tile(W_b.astype(np.float16)[None, :], (128, 1))
    blob[:, :, WBB_C:WBB_C + F // 2] = wbb.view(np.int32)[None]
    blob[:, 0:2, AR_C:AR_C + F] = a_w.reshape(2, F).view(np.int32)[None]
    blob[:, 0:2, WB2_C:WB2_C + F] = (
        np.tile(W_b[None, :], (2, 1)).view(np.int32)[None])
    return blob


def kernel(h, adj, W_w, W_b, a_w):
    h = np.ascontiguousarray(np.asarray(h, dtype=np.float32))
    adj = np.ascontiguousarray(np.asarray(adj, dtype=np.int32))
    W_w = np.ascontiguousarray(np.asarray(W_w, dtype=np.float32))
    W_b = np.ascontiguousarray(np.asarray(W_b, dtype=np.float32)).reshape(F)
    a_w = np.ascontiguousarray(np.asarray(a_w, dtype=np.float32)).reshape(2 * F)

    B = h.shape[0]
    assert B == N_CORES and h.shape == (B, N, F) and adj.shape == (B, N, N)

    blob = make_blobs(h, adj, W_w, W_b, a_w)
    nc = _get_nc()
    in_maps = [{"blob": blob[b]} for b in range(B)]
    res = run_bass_kernel_spmd(nc, in_maps, core_ids=list(range(N_CORES)))
    # device out is [o, i]; reference wants [i, o]
    return np.stack([
        res.results[b]["out"].T.astype(np.float32) for b in range(B)
    ], axis=0)
